# revision 37
# baseline (speedup 1.0000x reference)
"""MultiHeadAttention Trainium2 kernel (8 NeuronCores, data-parallel over batch).

Contract: kernel(**inputs) takes the FULL inputs from setup_inputs() and
returns the FULL [8, 512, 1024] output. Batch element c runs on NeuronCore c
(B == n_cores == 8); each core runs the same Bass/Tile program on its own
shard. No collectives.

Per-core computation (batch b, S=512, D=1024, H=16, Dk=64), all matmul
operands bf16 (fp32 PSUM accumulation), which halves HBM traffic and SBUF
footprint vs fp32r at the same PE rate:
  QT = (w_q/8)^T-proj of query^T  -> [D, S]  (query columns reversed)
  KT likewise (unscaled)          -> [D, S]
  V  = value proj + b_v           -> [128, sb, h, 65]  (col 64 = ones)
  per head h (chunk i = h//2, partitions p0 = (h%2)*64):
    scoresT[k,q'] = KT_h^T @ QT_h           (2 matmuls per [128,1024] PSUM)
    expS = exp(scoresT)                     (ACT, [128,1024] granularity)
    attn = expS * emt_h                     (DVE 2x bf16; emt = exp(bias)*mask
                                             precomputed on host, fp16)
    ctxT[65,S] = [V_h | 1]^T @ attn         (row 64 = softmax denominators)
    recip = 1/denom (DVE) -> bcast to 64 partitions (GpSimd) -> ctxT *= recip
  out_rev[q', e] = ctxT^T @ w_o^T + b_o ; host un-reverses rows.

The query-direction reversal makes rel_bias[k - q + 511, h] == rel_bias[k +
q', h], a positive-stride layout the host materializes (as exp) directly.
Bias adds ride the PSUM->SBUF drains: b_q/b_k as DVE tensor_scalar adds,
b_v/b_o as DVE tensor_tensor adds against partition-broadcast rows.
"""
import numpy as np
import ml_dtypes

import concourse.bass as bass
import concourse.tile as tile
from concourse import bacc, library_config, mybir
from concourse.bass_utils import run_bass_kernel_spmd

S = 512
D = 1024
H = 16
DK = 64
N_CORES = 8
NCH = D // 128  # 8 d-model chunks of 128
SB = S // 128   # 4 seq blocks of 128
F32 = mybir.dt.float32
BF16 = mybir.dt.bfloat16
F16 = mybir.dt.float16

BF = ml_dtypes.bfloat16

_CACHE = {}


def _build_program():
    nc = bacc.Bacc("TRN2", target_bir_lowering=False, debug=False,
                   num_devices=N_CORES)

    qT = nc.dram_tensor("qT", [D, S], BF16, kind="ExternalInput").ap()
    kT = nc.dram_tensor("kT", [D, S], BF16, kind="ExternalInput").ap()
    vT = nc.dram_tensor("vT", [D, S], BF16, kind="ExternalInput").ap()
    # emt grouped by head PAIR: emt[pr, k, hd, q'] = exp(bias+mask)[2pr+hd]
    emt = nc.dram_tensor("emt", [H // 2, S, 2, S], F16, kind="ExternalInput").ap()
    wq = nc.dram_tensor("wq", [D, D], BF16, kind="ExternalInput").ap()
    wk = nc.dram_tensor("wk", [D, D], BF16, kind="ExternalInput").ap()
    wv = nc.dram_tensor("wv", [D, D], BF16, kind="ExternalInput").ap()
    wo = nc.dram_tensor("wo", [D, D], BF16, kind="ExternalInput").ap()
    bq = nc.dram_tensor("bq", [128, NCH], F32, kind="ExternalInput").ap()
    bk = nc.dram_tensor("bk", [128, NCH], F32, kind="ExternalInput").ap()
    bve = nc.dram_tensor("bve", [1, D], BF16, kind="ExternalInput").ap()
    boe = nc.dram_tensor("boe", [1, D], BF16, kind="ExternalInput").ap()
    out = nc.dram_tensor("out", [S, D], BF16, kind="ExternalOutput").ap()

    qT3 = qT.rearrange("(c p) s -> p c s", p=128)      # [128, 8, 512]
    kT3 = kT.rearrange("(c p) s -> p c s", p=128)
    vT3 = vT.rearrange("(c p) s -> p c s", p=128)
    emt5 = emt.rearrange("r (kb p) hd q -> r p kb hd q", p=128)  # [8,128,4,2,512]
    wq3 = wq.rearrange("(c p) e -> c p e", p=128)      # [8, 128, 1024]
    wk3 = wk.rearrange("(c p) e -> c p e", p=128)
    wv3 = wv.rearrange("(c p) e -> c p e", p=128)
    wo3 = wo.rearrange("(c p) e -> c p e", p=128)
    out3 = out.rearrange("(sb p) e -> sb p e", p=128)  # [4, 128, 1024]

    from contextlib import ExitStack

    with tile.TileContext(nc) as tc, ExitStack() as ctx:
        singles = ctx.enter_context(tc.tile_pool(name="singles", bufs=1))
        wpool = ctx.enter_context(tc.tile_pool(name="wpool", bufs=4))
        emtpool = ctx.enter_context(tc.tile_pool(name="emtpool", bufs=2))
        exppool = ctx.enter_context(tc.tile_pool(name="exppool", bufs=2))
        attnpool = ctx.enter_context(tc.tile_pool(name="attnpool", bufs=3))
        smallpool = ctx.enter_context(tc.tile_pool(name="smallpool", bufs=1))
        outpool = ctx.enter_context(tc.tile_pool(name="outpool", bufs=6))
        ps_proj = ctx.enter_context(tc.tile_pool(name="ps_proj", bufs=2, space="PSUM"))
        ps_sc = ctx.enter_context(tc.tile_pool(name="ps_sc", bufs=2, space="PSUM"))
        ps_ctx = ctx.enter_context(tc.tile_pool(name="ps_ctx", bufs=2, space="PSUM"))

        # partition_broadcast is a GpSimd extended instruction (attn library)
        nc.gpsimd.load_library(library_config.attn)

        # ---- constants (scalar-engine DMA queue; keeps the sync queue free
        # for the bulk weight streams whose latency gates the PE) ----
        bq_sb = singles.tile([128, NCH], F32, tag="bq")
        bk_sb = singles.tile([128, NCH], F32, tag="bk")
        nc.scalar.dma_start(out=bq_sb, in_=bq)
        nc.scalar.dma_start(out=bk_sb, in_=bk)
        bve_sb = singles.tile([1, D], BF16, tag="bve")
        boe_sb = singles.tile([1, D], BF16, tag="boe")
        nc.scalar.dma_start(out=bve_sb, in_=bve)
        nc.scalar.dma_start(out=boe_sb, in_=boe)
        # full-K/full-M warmup operand: the PE activity monitor ignores
        # thin-K matmuls, so K=1 warm-up rows never un-throttle the clock
        ones2 = singles.tile([128, 512], BF16, tag="ones2")
        nc.vector.memset(ones2, 1.0)
        bve_bc = singles.tile([128, D], BF16, tag="bve_bc")
        boe_bc = singles.tile([128, D], BF16, tag="boe_bc")
        nc.gpsimd.partition_broadcast(bve_bc, bve_sb)
        nc.gpsimd.partition_broadcast(boe_bc, boe_sb)
        # pre-load the ACT exp table before the first real exp
        exp_warm = singles.tile([1, 32], F32, tag="exp_warm")
        nc.scalar.activation(exp_warm, ones2[0:1, 0:32],
                             mybir.ActivationFunctionType.Exp)

        # HAM warm-up: throwaway matmuls covering the wv+vT DMA window, so
        # the PE clock-gate reaches 8/8 before real work is ready and the PE
        # is never idle long enough to re-throttle. Operands are memset
        # on-chip so no DMA gates the first matmul.
        for _ in range(44):
            pd = ps_proj.tile([128, 512], F32, tag="proj")
            nc.tensor.matmul(pd, lhsT=ones2[:, :128], rhs=ones2,
                             start=True, stop=True)

        # ---- bulk loads: one DMA per matrix, split across the three DGE
        # queues so the pre-phase streams in parallel: sync takes wv/wq (+emt
        # later), scalar takes the small activations, gpsimd takes wk ----
        wq4 = wq.rearrange("(c p) e -> p c e", p=128)
        wk4 = wk.rearrange("(c p) e -> p c e", p=128)
        wv4 = wv.rearrange("(c p) e -> p c e", p=128)
        wo4 = wo.rearrange("(c p) e -> p c e", p=128)
        wv_t = wpool.tile([128, NCH, D], BF16, tag="w")
        nc.sync.dma_start(out=wv_t, in_=wv4)
        vT_sb = singles.tile([128, NCH, S], BF16, tag="vT")
        nc.scalar.dma_start(out=vT_sb, in_=vT3)
        qT_sb = singles.tile([128, NCH, S], BF16, tag="qT")
        nc.scalar.dma_start(out=qT_sb, in_=qT3)
        kT_sb = singles.tile([128, NCH, S], BF16, tag="kT")
        nc.scalar.dma_start(out=kT_sb, in_=kT3)
        wq_t = wpool.tile([128, NCH, D], BF16, tag="w")
        nc.sync.dma_start(out=wq_t, in_=wq4)
        wk_t = wpool.tile([128, NCH, D], BF16, tag="w")
        nc.gpsimd.dma_start(out=wk_t, in_=wk4)
        wv_sb = [wv_t[:, dc, :] for dc in range(NCH)]
        wq_sb = [wq_t[:, dc, :] for dc in range(NCH)]
        wk_sb = [wk_t[:, dc, :] for dc in range(NCH)]

        # emt per head-pair, also on the sync queue, emission interleaved
        # with the pair loop so slot-waits never delay anything time-critical
        emt_sb = {}

        def emit_emt(pr):
            if pr < H // 2:
                t = emtpool.tile([128, SB, 2, S], F16, tag="emt")
                nc.sync.dma_start(out=t, in_=emt5[pr])
                emt_sb[pr] = t

        for pr in range(2):
            emit_emt(pr)
        # wo after the first emt pairs: it isn't needed until out-proj, and
        # ahead of them it starves the first attention pairs of emt data
        wo_t = wpool.tile([128, NCH, D], BF16, tag="w")
        nc.sync.dma_start(out=wo_t, in_=wo4)
        wo_sb = [wo_t[:, dc, :] for dc in range(NCH)]

        # ---- persistent activations ----
        QT_sb = singles.tile([128, NCH, S], BF16, tag="QT")
        KT_sb = singles.tile([128, NCH, S], BF16, tag="KT")
        V_sb = singles.tile([128, SB, H, DK + 1], BF16, tag="V")
        ctxT_sb = singles.tile([128, NCH, S], BF16, tag="ctxT")
        nc.vector.memset(V_sb[:, :, :, DK:DK + 1], 1.0)

        # ---- V projection: V[s, e] = vT^T @ wv + b_v ----
        for sb in range(SB):
            for eh in range(2):
                pv = ps_proj.tile([128, 512], F32, tag="proj")
                for dc in range(NCH):
                    nc.tensor.matmul(
                        pv,
                        lhsT=vT_sb[:, dc, sb * 128:(sb + 1) * 128],
                        rhs=wv_sb[dc][:, eh * 512:(eh + 1) * 512],
                        start=(dc == 0), stop=(dc == NCH - 1),
                    )
                nc.vector.tensor_add(
                    V_sb[:, sb, 8 * eh:8 * eh + 8, 0:DK],
                    pv.rearrange("p (h d) -> p h d", d=DK),
                    bve_bc[:, eh * 512:(eh + 1) * 512].rearrange(
                        "p (h d) -> p h d", d=DK),
                )

        # ---- interleaved Q/K projection chunks + attention head pairs ----
        # Scores for heads (2i, 2i+1) run as concurrent row-tiled matmul
        # pairs: head 2i's K/Q slices sit at partitions 0-63 (PE row groups
        # 0-1), head 2i+1's at 64-127 (groups 2-3), writing the two bank
        # halves of one [128,1024] PSUM tile. Both stream the same QT columns
        # so the shared-XBUS pacing is trivially consistent.
        pair_state = {}

        def emit_ctx_head(h, attn_t):
            i, hd, p0 = h // 2, h % 2, (h % 2) * 64
            pc = ps_ctx.tile([DK + 1, 512], F32, tag="ctx")
            for kb in range(SB):
                nc.tensor.matmul(
                    pc, lhsT=V_sb[:, kb, h, :], rhs=attn_t[:, kb, hd, :],
                    start=(kb == 0), stop=(kb == SB - 1),
                )
            if hd == 0:
                sums_sb = smallpool.tile([1, 1024], F32, tag="sums")
                pair_state[i] = (sums_sb, pc)
            else:
                sums_sb, pc_a = pair_state.pop(i)
            # DVE custom reciprocal can't read PSUM; stage sums in SBUF
            nc.scalar.copy(sums_sb[:, hd * 512:(hd + 1) * 512],
                           pc[DK:DK + 1, :])
            if hd == 1:
                rc = smallpool.tile([1, 1024], F32, tag="rc")
                nc.vector.reciprocal_approx_fast(out=rc, in_=sums_sb)
                rbc = smallpool.tile([64, 1024], F32, tag="rbc")
                nc.gpsimd.partition_broadcast(rbc, rc)
                nc.vector.tensor_mul(ctxT_sb[0:64, i, :], pc_a[0:DK, :],
                                     rbc[:, 0:512])
                nc.vector.tensor_mul(ctxT_sb[64:128, i, :], pc[0:DK, :],
                                     rbc[:, 512:1024])

        pending = []

        def pop_pending():
            if pending:
                emit_ctx_head(*pending.pop(0))

        pq_of, pk_of = {}, {}

        def emit_proj_q_mms(i):
            if i >= NCH:
                return
            pq = ps_proj.tile([128, 512], F32, tag="proj", name=f"pq_{i}")
            for dc in range(NCH):
                nc.tensor.matmul(
                    pq, lhsT=wq_sb[dc][:, i * 128:(i + 1) * 128],
                    rhs=qT_sb[:, dc, :],
                    start=(dc == 0), stop=(dc == NCH - 1),
                )
            pq_of[i] = pq

        def emit_proj_q_add(i):
            # emitted late so the ACT FIFO keeps the exps back-to-back
            if i in pq_of:
                nc.scalar.add(QT_sb[:, i, :], pq_of.pop(i), bq_sb[:, i:i + 1])

        def emit_proj_k(i):
            if i >= NCH:
                return
            pk = ps_proj.tile([128, 512], F32, tag="proj", name=f"pk_{i}")
            for dc in range(NCH):
                nc.tensor.matmul(
                    pk, lhsT=wk_sb[dc][:, i * 128:(i + 1) * 128],
                    rhs=kT_sb[:, dc, :],
                    start=(dc == 0), stop=(dc == NCH - 1),
                )
            nc.vector.tensor_scalar_add(KT_sb[:, i, :], pk, bk_sb[:, i:i + 1])

        emit_proj_q_mms(0)
        emit_proj_q_add(0)
        emit_proj_k(0)
        for i in range(NCH):  # e-chunk i covers heads 2i, 2i+1
            exp_t = exppool.tile([128, SB, 2, S], BF16, tag="exp")
            attn_t = attnpool.tile([128, SB, 2, S], BF16, tag="attn")
            for kb in range(SB):
                psc = ps_sc.tile([128, 1024], F32, tag="sc")
                nc.tensor.matmul(
                    psc[:, 0:512],
                    lhsT=KT_sb[0:64, i, kb * 128:(kb + 1) * 128],
                    rhs=QT_sb[0:64, i, :], start=True, stop=True,
                )
                nc.tensor.matmul(
                    psc[:, 512:1024],
                    lhsT=KT_sb[64:128, i, kb * 128:(kb + 1) * 128],
                    rhs=QT_sb[64:128, i, :], start=True, stop=True,
                )
                nc.scalar.activation(
                    exp_t[:, kb, :, :],
                    psc.rearrange("p (hd q) -> p hd q", q=512),
                    mybir.ActivationFunctionType.Exp,
                )
                # The attn multiply is split in kb halves: a single 4096-wide
                # DVE op head-of-line-blocks the recip/norm chain ops behind
                # it in the strict DVE FIFO for ~3us, stalling ctx matmuls.
                # Next-chunk projection matmuls sit between the score bursts
                # so the PE never reaches the psc-slot reuse (kb 2/3) before
                # the exp drain of kb 0/1; their bias-add drains are deferred
                # so the ACT FIFO runs the four exps back-to-back.
                if kb == 1:
                    nc.vector.tensor_mul(attn_t[:, 0:2, :, :],
                                         exp_t[:, 0:2, :, :],
                                         emt_sb[i][:, 0:2, :, :])
                    emit_proj_q_mms(i + 1)
                elif kb == 2:
                    emit_proj_k(i + 1)
            emit_proj_q_add(i + 1)
            nc.vector.tensor_mul(attn_t[:, 2:4, :, :], exp_t[:, 2:4, :, :],
                                 emt_sb[i][:, 2:4, :, :])
            emit_emt(i + 2)
            pending.append((2 * i, attn_t))
            pending.append((2 * i + 1, attn_t))
            while len(pending) > 2:
                pop_pending()
        # ---- output projection: out_rev[q', e] = ctxT^T @ wo + b_o ----
        # Six blocks accumulate chunks 0..6 interleaved with the pending-ctx
        # drain (those chunks' heads are already normalized): two on the proj
        # psum pool, four on the now-idle scores pool ([128,1024] tiles hold
        # two blocks each). This keeps the PE densely busy across the
        # attention->out-proj seam instead of idling through the last heads'
        # softmax chains, whose norms gate only the final ch=7 matmuls.
        blocks = [(sb, eh) for sb in range(SB) for eh in range(2)]
        po_of = {}

        def alloc_out_psum():
            for pair_idx in range(2):
                t = ps_sc.tile([128, 1024], F32, tag="sc",
                               name=f"po_sc_{pair_idx}")
                po_of[blocks[2 + 2 * pair_idx]] = t[:, 0:512]
                po_of[blocks[3 + 2 * pair_idx]] = t[:, 512:1024]

        def emit_out_mms(blk, ch_range, start, stop):
            sb, eh = blk
            if blk not in po_of:
                po_of[blk] = ps_proj.tile([128, 512], F32, tag="proj",
                                          name=f"po_{sb}_{eh}")
            po = po_of[blk]
            for ch in ch_range:
                nc.tensor.matmul(
                    po, lhsT=ctxT_sb[:, ch, sb * 128:(sb + 1) * 128],
                    rhs=wo_sb[ch][:, eh * 512:(eh + 1) * 512],
                    start=(start and ch == ch_range[0]),
                    stop=(stop and ch == ch_range[-1]),
                )

        def emit_out_drain(blk, engine):
            sb, eh = blk
            po = po_of.pop(blk)
            osb = outpool.tile([128, 512], BF16, tag="out")
            nc.vector.tensor_add(osb, po, boe_bc[:, eh * 512:(eh + 1) * 512])
            # alternate the two HWDGE rings: eight back-to-back stores on one
            # ring serialize on per-DMA completion latency at the very tail
            engine.dma_start(
                out=out3[sb, :, eh * 512:(eh + 1) * 512], in_=osb)

        assert [h for h, _ in pending] == [14, 15], pending
        # both ctx heads first: head 15's softmax chain gates the final ch=7
        # matmuls, so it must not queue behind the partial-accumulation mms
        pop_pending()                           # head 14
        pop_pending()                           # head 15 (chunk 7 completes)
        emit_out_mms(blocks[0], range(0, 7), start=True, stop=False)
        emit_out_mms(blocks[1], range(0, 7), start=True, stop=False)
        alloc_out_psum()
        for blk in blocks[2:6]:
            emit_out_mms(blk, range(0, 7), start=True, stop=False)
        # proj-pool blocks finish + drain first so b6/b7 can reuse their psum
        for n, blk in enumerate(blocks[:2]):
            emit_out_mms(blk, range(7, 8), start=False, stop=True)
            emit_out_drain(blk, nc.sync if n % 2 else nc.scalar)
        for n, blk in enumerate(blocks[6:]):
            emit_out_mms(blk, range(NCH), start=True, stop=True)
            emit_out_drain(blk, nc.sync if n % 2 else nc.scalar)
        for n, blk in enumerate(blocks[2:6]):
            emit_out_mms(blk, range(7, 8), start=False, stop=True)
            emit_out_drain(blk, nc.sync if n % 2 else nc.scalar)

    nc.compile()
    return nc


def _prep_inputs(query, key, value, mask, w_q, b_q, w_k, b_k, w_v, b_v,
                 w_o, b_o, rel_bias):
    query = np.asarray(query, np.float32)
    key = np.asarray(key, np.float32)
    value = np.asarray(value, np.float32)
    mask = np.asarray(mask)
    w_q = np.asarray(w_q, np.float32)
    w_k = np.asarray(w_k, np.float32)
    w_v = np.asarray(w_v, np.float32)
    w_o = np.asarray(w_o, np.float32)
    b_q = np.asarray(b_q, np.float32)
    b_k = np.asarray(b_k, np.float32)
    b_v = np.asarray(b_v, np.float32)
    b_o = np.asarray(b_o, np.float32)
    rel_bias = np.asarray(rel_bias, np.float32)

    shared = {
        "wq": np.ascontiguousarray((w_q.T / 8.0).astype(BF)),
        "wk": np.ascontiguousarray(w_k.T.astype(BF)),
        "wv": np.ascontiguousarray(w_v.T.astype(BF)),
        "wo": np.ascontiguousarray(w_o.T.astype(BF)),
        "bq": np.ascontiguousarray((b_q / 8.0).reshape(NCH, 128).T),
        "bk": np.ascontiguousarray(b_k.reshape(NCH, 128).T),
        "bve": b_v.reshape(1, D).astype(BF),
        "boe": b_o.reshape(1, D).astype(BF),
    }

    # ebias[h, k, q'] = exp(rel_bias[k + q', h]) ; masked entries -> 0
    idx = np.arange(S)[:, None] + np.arange(S)[None, :]   # [k, q'] in [0, 1022]
    ebias = np.exp(rel_bias[idx])                          # [S, S, H]
    ebias = np.ascontiguousarray(ebias.transpose(2, 0, 1))  # [H, k, q']

    in_maps = []
    for c in range(N_CORES):
        m01 = mask[c, 0][::-1, :].T.astype(np.float32)     # [k, q'] in {0,1}
        emt = (ebias * m01[None]).astype(np.float16)       # [H, k, q']
        # group by head pair: emt_p[pr, k, hd, q'] = emt[2*pr + hd, k, q']
        emt_p = emt.reshape(H // 2, 2, S, S).transpose(0, 2, 1, 3)
        im = dict(shared)
        im["qT"] = np.ascontiguousarray(query[c].T[:, ::-1].astype(BF))
        im["kT"] = np.ascontiguousarray(key[c].T.astype(BF))
        im["vT"] = np.ascontiguousarray(value[c].T.astype(BF))
        im["emt"] = np.ascontiguousarray(emt_p)
        in_maps.append(im)
    return in_maps


def kernel(query, key, value, mask, w_q, b_q, w_k, b_k, w_v, b_v, w_o, b_o,
           rel_bias, _run_opts=None):
    if "nc" not in _CACHE:
        _CACHE["nc"] = _build_program()
    nc = _CACHE["nc"]
    in_maps = _prep_inputs(query, key, value, mask, w_q, b_q, w_k, b_k,
                           w_v, b_v, w_o, b_o, rel_bias)
    opts = _run_opts or {}
    res = run_bass_kernel_spmd(nc, in_maps, list(range(N_CORES)), **opts)
    out = np.stack([np.asarray(res.results[c]["out"], np.float32)[::-1, :]
                    for c in range(N_CORES)])
    if _run_opts is not None:
        _CACHE["last_result"] = res
    return out


# revision 38
# speedup vs baseline: 1.1119x; 1.1119x over previous
"""MultiHeadAttention Trainium2 kernel (8 NeuronCores, data-parallel over batch).

Contract: kernel(**inputs) takes the FULL inputs from setup_inputs() and
returns the FULL [8, 512, 1024] output. Batch element c runs on NeuronCore c
(B == n_cores == 8); each core runs the same Bass/Tile program on its own
shard. No collectives.

Per-core computation (batch b, S=512, D=1024, H=16, Dk=64), all matmul
operands bf16 (fp32 PSUM accumulation), which halves HBM traffic and SBUF
footprint vs fp32r at the same PE rate:
  QT = (w_q/8)^T-proj of query^T  -> [D, S]  (query columns reversed)
  KT likewise (unscaled)          -> [D, S]
  V  = value proj + b_v           -> [128, sb, h, 65]  (col 64 = ones)
  per head h (chunk i = h//2, partitions p0 = (h%2)*64):
    scoresT[k,q'] = KT_h^T @ QT_h           (2 matmuls per [128,1024] PSUM)
    expS = exp(scoresT)                     (ACT, [128,1024] granularity)
    attn = expS * emt_h                     (DVE 2x bf16; emt = exp(bias)*mask
                                             precomputed on host, fp16)
    ctxT[65,S] = [V_h | 1]^T @ attn         (row 64 = softmax denominators)
    recip = 1/denom (DVE) -> bcast to 64 partitions (GpSimd) -> ctxT *= recip
  out_rev[q', e] = ctxT^T @ w_o^T + b_o ; host un-reverses rows.

The query-direction reversal makes rel_bias[k - q + 511, h] == rel_bias[k +
q', h], a positive-stride layout the host materializes (as exp) directly.
Bias adds ride the PSUM->SBUF drains: b_q/b_k as DVE tensor_scalar adds,
b_v/b_o as DVE tensor_tensor adds against partition-broadcast rows.
"""
import numpy as np
import ml_dtypes

import concourse.bass as bass
import concourse.tile as tile
from concourse import bacc, library_config, mybir
from concourse.bass_utils import run_bass_kernel_spmd

S = 512
D = 1024
H = 16
DK = 64
N_CORES = 8
NCH = D // 128  # 8 d-model chunks of 128
SB = S // 128   # 4 seq blocks of 128
F32 = mybir.dt.float32
BF16 = mybir.dt.bfloat16
F16 = mybir.dt.float16

BF = ml_dtypes.bfloat16

_CACHE = {}


def _build_program():
    nc = bacc.Bacc("TRN2", target_bir_lowering=False, debug=False,
                   num_devices=N_CORES)

    qT = nc.dram_tensor("qT", [D, S], BF16, kind="ExternalInput").ap()
    kT = nc.dram_tensor("kT", [D, S], BF16, kind="ExternalInput").ap()
    vT = nc.dram_tensor("vT", [D, S], BF16, kind="ExternalInput").ap()
    # emt grouped by head PAIR: emt[pr, k, hd, q'] = exp(bias+mask)[2pr+hd]
    emt = nc.dram_tensor("emt", [H // 2, S, 2, S], F16, kind="ExternalInput").ap()
    wq = nc.dram_tensor("wq", [D, D], BF16, kind="ExternalInput").ap()
    wk = nc.dram_tensor("wk", [D, D], BF16, kind="ExternalInput").ap()
    wv = nc.dram_tensor("wv", [D, D], BF16, kind="ExternalInput").ap()
    wo = nc.dram_tensor("wo", [D, D], BF16, kind="ExternalInput").ap()
    bq = nc.dram_tensor("bq", [128, NCH], F32, kind="ExternalInput").ap()
    bk = nc.dram_tensor("bk", [128, NCH], F32, kind="ExternalInput").ap()
    bve = nc.dram_tensor("bve", [1, D], BF16, kind="ExternalInput").ap()
    boe = nc.dram_tensor("boe", [1, D], BF16, kind="ExternalInput").ap()
    out = nc.dram_tensor("out", [S, D], BF16, kind="ExternalOutput").ap()

    qT3 = qT.rearrange("(c p) s -> p c s", p=128)      # [128, 8, 512]
    kT3 = kT.rearrange("(c p) s -> p c s", p=128)
    vT3 = vT.rearrange("(c p) s -> p c s", p=128)
    emt5 = emt.rearrange("r (kb p) hd q -> r p kb hd q", p=128)  # [8,128,4,2,512]
    wq3 = wq.rearrange("(c p) e -> c p e", p=128)      # [8, 128, 1024]
    wk3 = wk.rearrange("(c p) e -> c p e", p=128)
    wv3 = wv.rearrange("(c p) e -> c p e", p=128)
    wo3 = wo.rearrange("(c p) e -> c p e", p=128)
    out3 = out.rearrange("(sb p) e -> sb p e", p=128)  # [4, 128, 1024]

    from contextlib import ExitStack

    with tile.TileContext(nc) as tc, ExitStack() as ctx:
        singles = ctx.enter_context(tc.tile_pool(name="singles", bufs=1))
        wpool = ctx.enter_context(tc.tile_pool(name="wpool", bufs=4))
        emtpool = ctx.enter_context(tc.tile_pool(name="emtpool", bufs=2))
        exppool = ctx.enter_context(tc.tile_pool(name="exppool", bufs=2))
        attnpool = ctx.enter_context(tc.tile_pool(name="attnpool", bufs=3))
        smallpool = ctx.enter_context(tc.tile_pool(name="smallpool", bufs=1))
        outpool = ctx.enter_context(tc.tile_pool(name="outpool", bufs=6))
        ps_proj = ctx.enter_context(tc.tile_pool(name="ps_proj", bufs=2, space="PSUM"))
        ps_sc = ctx.enter_context(tc.tile_pool(name="ps_sc", bufs=2, space="PSUM"))
        ps_ctx = ctx.enter_context(tc.tile_pool(name="ps_ctx", bufs=2, space="PSUM"))

        # partition_broadcast is a GpSimd extended instruction (attn library)
        nc.gpsimd.load_library(library_config.attn)

        # ---- constants (scalar-engine DMA queue; keeps the sync queue free
        # for the bulk weight streams whose latency gates the PE) ----
        bq_sb = singles.tile([128, NCH], F32, tag="bq")
        bk_sb = singles.tile([128, NCH], F32, tag="bk")
        nc.scalar.dma_start(out=bq_sb, in_=bq)
        nc.scalar.dma_start(out=bk_sb, in_=bk)
        bve_sb = singles.tile([1, D], BF16, tag="bve")
        boe_sb = singles.tile([1, D], BF16, tag="boe")
        nc.scalar.dma_start(out=bve_sb, in_=bve)
        nc.scalar.dma_start(out=boe_sb, in_=boe)
        # full-K/full-M warmup operand: the PE activity monitor ignores
        # thin-K matmuls, so K=1 warm-up rows never un-throttle the clock
        ones2 = singles.tile([128, 512], BF16, tag="ones2")
        nc.vector.memset(ones2, 1.0)
        bve_bc = singles.tile([128, D], BF16, tag="bve_bc")
        boe_bc = singles.tile([128, D], BF16, tag="boe_bc")
        nc.gpsimd.partition_broadcast(bve_bc, bve_sb)
        nc.gpsimd.partition_broadcast(boe_bc, boe_sb)
        # pre-load the ACT exp table before the first real exp
        exp_warm = singles.tile([1, 32], F32, tag="exp_warm")
        nc.scalar.activation(exp_warm, ones2[0:1, 0:32],
                             mybir.ActivationFunctionType.Exp)

        # HAM warm-up: throwaway matmuls covering the wv+vT DMA window, so
        # the PE clock-gate reaches 8/8 before real work is ready and the PE
        # is never idle long enough to re-throttle. Operands are memset
        # on-chip so no DMA gates the first matmul.
        for _ in range(56):
            pd = ps_proj.tile([128, 512], F32, tag="proj")
            nc.tensor.matmul(pd, lhsT=ones2[:, :128], rhs=ones2,
                             start=True, stop=True)

        # ---- bulk loads: one DMA per matrix, in consumption order, all on
        # the sync queue (splitting across rings slows the earliest stream;
        # measured: wk via gpsimd SWDGE took 37us and attention started 7us
        # later) ----
        wq4 = wq.rearrange("(c p) e -> p c e", p=128)
        wk4 = wk.rearrange("(c p) e -> p c e", p=128)
        wv4 = wv.rearrange("(c p) e -> p c e", p=128)
        wo4 = wo.rearrange("(c p) e -> p c e", p=128)
        wv_t = wpool.tile([128, NCH, D], BF16, tag="w")
        nc.sync.dma_start(out=wv_t, in_=wv4)
        vT_sb = singles.tile([128, NCH, S], BF16, tag="vT")
        nc.sync.dma_start(out=vT_sb, in_=vT3)
        wq_t = wpool.tile([128, NCH, D], BF16, tag="w")
        nc.sync.dma_start(out=wq_t, in_=wq4)
        qT_sb = singles.tile([128, NCH, S], BF16, tag="qT")
        nc.sync.dma_start(out=qT_sb, in_=qT3)
        wk_t = wpool.tile([128, NCH, D], BF16, tag="w")
        nc.sync.dma_start(out=wk_t, in_=wk4)
        kT_sb = singles.tile([128, NCH, S], BF16, tag="kT")
        nc.sync.dma_start(out=kT_sb, in_=kT3)
        wv_sb = [wv_t[:, dc, :] for dc in range(NCH)]
        wq_sb = [wq_t[:, dc, :] for dc in range(NCH)]
        wk_sb = [wk_t[:, dc, :] for dc in range(NCH)]

        # emt per head-pair, also on the sync queue, emission interleaved
        # with the pair loop so slot-waits never delay anything time-critical
        emt_sb = {}

        def emit_emt(pr):
            if pr < H // 2:
                t = emtpool.tile([128, SB, 2, S], F16, tag="emt")
                nc.sync.dma_start(out=t, in_=emt5[pr])
                emt_sb[pr] = t

        for pr in range(2):
            emit_emt(pr)
        # wo after the first emt pairs: it isn't needed until out-proj, and
        # ahead of them it starves the first attention pairs of emt data
        wo_t = wpool.tile([128, NCH, D], BF16, tag="w")
        nc.sync.dma_start(out=wo_t, in_=wo4)
        wo_sb = [wo_t[:, dc, :] for dc in range(NCH)]

        # ---- persistent activations ----
        QT_sb = singles.tile([128, NCH, S], BF16, tag="QT")
        KT_sb = singles.tile([128, NCH, S], BF16, tag="KT")
        V_sb = singles.tile([128, SB, H, DK + 1], BF16, tag="V")
        ctxT_sb = singles.tile([128, NCH, S], BF16, tag="ctxT")
        nc.vector.memset(V_sb[:, :, :, DK:DK + 1], 1.0)

        # ---- V projection: V[s, e] = vT^T @ wv + b_v ----
        for sb in range(SB):
            for eh in range(2):
                pv = ps_proj.tile([128, 512], F32, tag="proj")
                for dc in range(NCH):
                    nc.tensor.matmul(
                        pv,
                        lhsT=vT_sb[:, dc, sb * 128:(sb + 1) * 128],
                        rhs=wv_sb[dc][:, eh * 512:(eh + 1) * 512],
                        start=(dc == 0), stop=(dc == NCH - 1),
                    )
                nc.vector.tensor_add(
                    V_sb[:, sb, 8 * eh:8 * eh + 8, 0:DK],
                    pv.rearrange("p (h d) -> p h d", d=DK),
                    bve_bc[:, eh * 512:(eh + 1) * 512].rearrange(
                        "p (h d) -> p h d", d=DK),
                )

        # ---- interleaved Q/K projection chunks + attention head pairs ----
        # Scores for heads (2i, 2i+1) run as concurrent row-tiled matmul
        # pairs: head 2i's K/Q slices sit at partitions 0-63 (PE row groups
        # 0-1), head 2i+1's at 64-127 (groups 2-3), writing the two bank
        # halves of one [128,1024] PSUM tile. Both stream the same QT columns
        # so the shared-XBUS pacing is trivially consistent.
        pair_state = {}

        def emit_ctx_head(h, attn_t):
            i, hd, p0 = h // 2, h % 2, (h % 2) * 64
            pc = ps_ctx.tile([DK + 1, 512], F32, tag="ctx")
            for kb in range(SB):
                nc.tensor.matmul(
                    pc, lhsT=V_sb[:, kb, h, :], rhs=attn_t[:, kb, hd, :],
                    start=(kb == 0), stop=(kb == SB - 1),
                )
            if hd == 0:
                sums_sb = smallpool.tile([1, 1024], F32, tag="sums")
                pair_state[i] = (sums_sb, pc)
            else:
                sums_sb, pc_a = pair_state.pop(i)
            # DVE custom reciprocal can't read PSUM; stage sums in SBUF
            nc.scalar.copy(sums_sb[:, hd * 512:(hd + 1) * 512],
                           pc[DK:DK + 1, :])
            if hd == 1:
                rc = smallpool.tile([1, 1024], F32, tag="rc")
                nc.vector.reciprocal_approx_fast(out=rc, in_=sums_sb)
                rbc = smallpool.tile([64, 1024], F32, tag="rbc")
                nc.gpsimd.partition_broadcast(rbc, rc)
                nc.vector.tensor_mul(ctxT_sb[0:64, i, :], pc_a[0:DK, :],
                                     rbc[:, 0:512])
                nc.vector.tensor_mul(ctxT_sb[64:128, i, :], pc[0:DK, :],
                                     rbc[:, 512:1024])

        pending = []

        def pop_pending():
            if pending:
                emit_ctx_head(*pending.pop(0))

        pq_of, pk_of = {}, {}

        def emit_proj_q_mms(i):
            if i >= NCH:
                return
            pq = ps_proj.tile([128, 512], F32, tag="proj", name=f"pq_{i}")
            for dc in range(NCH):
                nc.tensor.matmul(
                    pq, lhsT=wq_sb[dc][:, i * 128:(i + 1) * 128],
                    rhs=qT_sb[:, dc, :],
                    start=(dc == 0), stop=(dc == NCH - 1),
                )
            pq_of[i] = pq

        def emit_proj_q_add(i):
            # emitted late so the ACT FIFO keeps the exps back-to-back
            if i in pq_of:
                nc.scalar.add(QT_sb[:, i, :], pq_of.pop(i), bq_sb[:, i:i + 1])

        def emit_proj_k(i):
            if i >= NCH:
                return
            pk = ps_proj.tile([128, 512], F32, tag="proj", name=f"pk_{i}")
            for dc in range(NCH):
                nc.tensor.matmul(
                    pk, lhsT=wk_sb[dc][:, i * 128:(i + 1) * 128],
                    rhs=kT_sb[:, dc, :],
                    start=(dc == 0), stop=(dc == NCH - 1),
                )
            nc.vector.tensor_scalar_add(KT_sb[:, i, :], pk, bk_sb[:, i:i + 1])

        emit_proj_q_mms(0)
        emit_proj_q_add(0)
        emit_proj_k(0)
        for i in range(NCH):  # e-chunk i covers heads 2i, 2i+1
            exp_t = exppool.tile([128, SB, 2, S], BF16, tag="exp")
            attn_t = attnpool.tile([128, SB, 2, S], BF16, tag="attn")
            for kb in range(SB):
                psc = ps_sc.tile([128, 1024], F32, tag="sc")
                nc.tensor.matmul(
                    psc[:, 0:512],
                    lhsT=KT_sb[0:64, i, kb * 128:(kb + 1) * 128],
                    rhs=QT_sb[0:64, i, :], start=True, stop=True,
                )
                nc.tensor.matmul(
                    psc[:, 512:1024],
                    lhsT=KT_sb[64:128, i, kb * 128:(kb + 1) * 128],
                    rhs=QT_sb[64:128, i, :], start=True, stop=True,
                )
                nc.scalar.activation(
                    exp_t[:, kb, :, :],
                    psc.rearrange("p (hd q) -> p hd q", q=512),
                    mybir.ActivationFunctionType.Exp,
                )
                # The attn multiply is split in kb halves: a single 4096-wide
                # DVE op head-of-line-blocks the recip/norm chain ops behind
                # it in the strict DVE FIFO for ~3us, stalling ctx matmuls.
                # Next-chunk projection matmuls sit between the score bursts
                # so the PE never reaches the psc-slot reuse (kb 2/3) before
                # the exp drain of kb 0/1; their bias-add drains are deferred
                # so the ACT FIFO runs the four exps back-to-back.
                if kb == 1:
                    nc.vector.tensor_mul(attn_t[:, 0:2, :, :],
                                         exp_t[:, 0:2, :, :],
                                         emt_sb[i][:, 0:2, :, :])
                    emit_proj_q_mms(i + 1)
                elif kb == 2:
                    emit_proj_k(i + 1)
            emit_proj_q_add(i + 1)
            nc.vector.tensor_mul(attn_t[:, 2:4, :, :], exp_t[:, 2:4, :, :],
                                 emt_sb[i][:, 2:4, :, :])
            emit_emt(i + 2)
            pending.append((2 * i, attn_t))
            pending.append((2 * i + 1, attn_t))
            while len(pending) > 2:
                pop_pending()
        # ---- output projection: out_rev[q', e] = ctxT^T @ wo + b_o ----
        # Six blocks accumulate chunks 0..6 interleaved with the pending-ctx
        # drain (those chunks' heads are already normalized): two on the proj
        # psum pool, four on the now-idle scores pool ([128,1024] tiles hold
        # two blocks each). This keeps the PE densely busy across the
        # attention->out-proj seam instead of idling through the last heads'
        # softmax chains, whose norms gate only the final ch=7 matmuls.
        blocks = [(sb, eh) for sb in range(SB) for eh in range(2)]
        po_of = {}

        def alloc_out_psum():
            for pair_idx in range(2):
                t = ps_sc.tile([128, 1024], F32, tag="sc",
                               name=f"po_sc_{pair_idx}")
                po_of[blocks[2 + 2 * pair_idx]] = t[:, 0:512]
                po_of[blocks[3 + 2 * pair_idx]] = t[:, 512:1024]

        def emit_out_mms(blk, ch_range, start, stop):
            sb, eh = blk
            if blk not in po_of:
                po_of[blk] = ps_proj.tile([128, 512], F32, tag="proj",
                                          name=f"po_{sb}_{eh}")
            po = po_of[blk]
            for ch in ch_range:
                nc.tensor.matmul(
                    po, lhsT=ctxT_sb[:, ch, sb * 128:(sb + 1) * 128],
                    rhs=wo_sb[ch][:, eh * 512:(eh + 1) * 512],
                    start=(start and ch == ch_range[0]),
                    stop=(stop and ch == ch_range[-1]),
                )

        def emit_out_drain(blk, engine):
            sb, eh = blk
            po = po_of.pop(blk)
            osb = outpool.tile([128, 512], BF16, tag="out")
            nc.vector.tensor_add(osb, po, boe_bc[:, eh * 512:(eh + 1) * 512])
            # alternate the two HWDGE rings: eight back-to-back stores on one
            # ring serialize on per-DMA completion latency at the very tail
            engine.dma_start(
                out=out3[sb, :, eh * 512:(eh + 1) * 512], in_=osb)

        assert [h for h, _ in pending] == [14, 15], pending
        # both ctx heads first: head 15's softmax chain gates the final ch=7
        # matmuls, so it must not queue behind the partial-accumulation mms
        pop_pending()                           # head 14
        pop_pending()                           # head 15 (chunk 7 completes)
        emit_out_mms(blocks[0], range(0, 7), start=True, stop=False)
        emit_out_mms(blocks[1], range(0, 7), start=True, stop=False)
        alloc_out_psum()
        for blk in blocks[2:6]:
            emit_out_mms(blk, range(0, 7), start=True, stop=False)
        # proj-pool blocks finish + drain first so b6/b7 can reuse their psum
        for n, blk in enumerate(blocks[:2]):
            emit_out_mms(blk, range(7, 8), start=False, stop=True)
            emit_out_drain(blk, nc.sync if n % 2 else nc.scalar)
        for n, blk in enumerate(blocks[6:]):
            emit_out_mms(blk, range(NCH), start=True, stop=True)
            emit_out_drain(blk, nc.sync if n % 2 else nc.scalar)
        for n, blk in enumerate(blocks[2:6]):
            emit_out_mms(blk, range(7, 8), start=False, stop=True)
            emit_out_drain(blk, nc.sync if n % 2 else nc.scalar)

    nc.compile()
    return nc


def _prep_inputs(query, key, value, mask, w_q, b_q, w_k, b_k, w_v, b_v,
                 w_o, b_o, rel_bias):
    query = np.asarray(query, np.float32)
    key = np.asarray(key, np.float32)
    value = np.asarray(value, np.float32)
    mask = np.asarray(mask)
    w_q = np.asarray(w_q, np.float32)
    w_k = np.asarray(w_k, np.float32)
    w_v = np.asarray(w_v, np.float32)
    w_o = np.asarray(w_o, np.float32)
    b_q = np.asarray(b_q, np.float32)
    b_k = np.asarray(b_k, np.float32)
    b_v = np.asarray(b_v, np.float32)
    b_o = np.asarray(b_o, np.float32)
    rel_bias = np.asarray(rel_bias, np.float32)

    shared = {
        "wq": np.ascontiguousarray((w_q.T / 8.0).astype(BF)),
        "wk": np.ascontiguousarray(w_k.T.astype(BF)),
        "wv": np.ascontiguousarray(w_v.T.astype(BF)),
        "wo": np.ascontiguousarray(w_o.T.astype(BF)),
        "bq": np.ascontiguousarray((b_q / 8.0).reshape(NCH, 128).T),
        "bk": np.ascontiguousarray(b_k.reshape(NCH, 128).T),
        "bve": b_v.reshape(1, D).astype(BF),
        "boe": b_o.reshape(1, D).astype(BF),
    }

    # ebias[h, k, q'] = exp(rel_bias[k + q', h]) ; masked entries -> 0
    idx = np.arange(S)[:, None] + np.arange(S)[None, :]   # [k, q'] in [0, 1022]
    ebias = np.exp(rel_bias[idx])                          # [S, S, H]
    ebias = np.ascontiguousarray(ebias.transpose(2, 0, 1))  # [H, k, q']

    in_maps = []
    for c in range(N_CORES):
        m01 = mask[c, 0][::-1, :].T.astype(np.float32)     # [k, q'] in {0,1}
        emt = (ebias * m01[None]).astype(np.float16)       # [H, k, q']
        # group by head pair: emt_p[pr, k, hd, q'] = emt[2*pr + hd, k, q']
        emt_p = emt.reshape(H // 2, 2, S, S).transpose(0, 2, 1, 3)
        im = dict(shared)
        im["qT"] = np.ascontiguousarray(query[c].T[:, ::-1].astype(BF))
        im["kT"] = np.ascontiguousarray(key[c].T.astype(BF))
        im["vT"] = np.ascontiguousarray(value[c].T.astype(BF))
        im["emt"] = np.ascontiguousarray(emt_p)
        in_maps.append(im)
    return in_maps


def kernel(query, key, value, mask, w_q, b_q, w_k, b_k, w_v, b_v, w_o, b_o,
           rel_bias, _run_opts=None):
    if "nc" not in _CACHE:
        _CACHE["nc"] = _build_program()
    nc = _CACHE["nc"]
    in_maps = _prep_inputs(query, key, value, mask, w_q, b_q, w_k, b_k,
                           w_v, b_v, w_o, b_o, rel_bias)
    opts = _run_opts or {}
    res = run_bass_kernel_spmd(nc, in_maps, list(range(N_CORES)), **opts)
    out = np.stack([np.asarray(res.results[c]["out"], np.float32)[::-1, :]
                    for c in range(N_CORES)])
    if _run_opts is not None:
        _CACHE["last_result"] = res
    return out
